# revision 6
# baseline (speedup 1.0000x reference)
"""Trainium2 Bass kernel for ArrowLoraLinearLayer (MoE top-2 LoRA routing).

Math (per token t):
  sim[t,e]  = |x[t,:] @ protos[e,:]|                      (E=8 experts)
  coeff     = softmax over top-2 of sim (others 0)
  z[t,:]    = x[t,:] @ A_all.T          A_all = [E*r, F]  (E*r = 128)
  W[er,t]   = coeff[t,e(er)] * z[t,er]
  out[t,:]  = W[:,t].T @ BT             BT[er,:] = scales[e] * B_stack[e,:,j].T

Sharding: data-parallel over tokens, 1024 tokens per core x 8 cores.
All weights replicated. No collectives.
"""

import sys
import types

sys.path.insert(0, "/opt/trn_rl_repo")

import numpy as np


def _install_ntff_hook_shim():
    """The agent image's antenv lacks axon_hooks; provide it so
    run_bass_kernel_spmd(trace=True) can profile via the axon .so."""
    if "antenv.axon_hooks" in sys.modules:
        return
    mod = types.ModuleType("antenv.axon_hooks")
    state = {"hook": None}

    def set_axon_ntff_profile_hook(h):
        state["hook"] = h

    def get_axon_ntff_profile_hook():
        if state["hook"] is None:
            try:
                from trn_agent_boot.trn_boot import _ntff_profile_via_ctypes

                state["hook"] = _ntff_profile_via_ctypes(
                    "/opt/axon/libaxon_pjrt.so"
                )
            except Exception:
                return None
        return state["hook"]

    mod.set_axon_ntff_profile_hook = set_axon_ntff_profile_hook
    mod.get_axon_ntff_profile_hook = get_axon_ntff_profile_hook
    sys.modules["antenv.axon_hooks"] = mod


_install_ntff_hook_shim()

import concourse.bass as bass
import concourse.mybir as mybir
from concourse.bass_utils import run_bass_kernel_spmd
from concourse.masks import make_identity
from concourse.tile import TileContext

def _split_multi_waits(nc, skip_opcodes=()):
    """Walrus allows only one sync-wait per engine instruction (e.g. the
    Matmult LDWEIGHTS slot, DMA_DIRECT2D). Move extra waits onto freshly
    inserted same-engine NoOps just before the instruction."""
    counter = 0
    for f in nc.m.functions:
        for b in f.blocks:
            il = b.instructions
            i = 0
            while i < len(il):
                inst = il[i]
                si = getattr(inst, "sync_info", None)
                if (
                    si is not None
                    and getattr(inst, "opcode", None) not in skip_opcodes
                    and len(si.on_wait) >= 2
                ):
                    waits = list(si.on_wait)
                    for w in waits:
                        nop = mybir.InstNoOp(name=f"I-waitsplit-{counter}")
                        counter += 1
                        nop.engine = inst.engine
                        nop.sync_info = mybir.SyncInfo(on_wait=[w], on_update=[])
                        il.insert(i, nop)
                        i += 1
                    inst.sync_info = mybir.SyncInfo(
                        on_wait=[], on_update=si.on_update
                    )
                i += 1


N_CORES = 8
P = 128            # partitions
F = 2048           # in features
O = 2048           # out features
E = 8              # experts
R = 16             # lora rank
ER = E * R         # 128
T_SHARD = 1024     # tokens per core
N_TILES = T_SHARD // P   # 8 token tiles per core
N_CHUNKS = F // P        # 16 K-chunks
FP = mybir.dt.float32

AF = mybir.ActivationFunctionType
ALU = mybir.AluOpType
AX = mybir.AxisListType


def build_nc():
    nc = bass.Bass(target_bir_lowering=False)

    x_ext = nc.declare_dram_parameter("x", [T_SHARD, F], FP, isOutput=False)
    at_ext = nc.declare_dram_parameter("at", [P, N_CHUNKS * P], FP, isOutput=False)
    pt_ext = nc.declare_dram_parameter("pt", [P, N_CHUNKS * E], FP, isOutput=False)
    bt_ext = nc.declare_dram_parameter("bt", [ER, O], FP, isOutput=False)
    sel_ext = nc.declare_dram_parameter("sel", [E, ER], FP, isOutput=False)
    out_ext = nc.declare_dram_parameter("out", [T_SHARD, O], FP, isOutput=True)

    with TileContext(nc) as tc:
        with (
            tc.tile_pool(name="const", bufs=1) as const,
            tc.tile_pool(name="xin", bufs=3) as xin_pool,
            tc.tile_pool(name="xt", bufs=1) as xt_pool,
            tc.tile_pool(name="rt", bufs=2) as rt_pool,
            tc.tile_pool(name="outp", bufs=2) as out_pool,
            tc.tile_pool(name="tpd", bufs=2, space="PSUM") as tpd_psum,
            tc.tile_pool(name="zp", bufs=1, space="PSUM") as z_pool,
            tc.tile_pool(name="sp", bufs=1, space="PSUM") as s_pool,
            tc.tile_pool(name="smallp", bufs=2, space="PSUM") as small_psum,
        ):
            ident = const.tile([P, P], FP)
            make_identity(nc, ident)

            at_sb = const.tile([P, N_CHUNKS * P], FP)
            nc.sync.dma_start(out=at_sb[:], in_=at_ext[:])
            pt_sb = const.tile([P, N_CHUNKS * E], FP)
            nc.sync.dma_start(out=pt_sb[:], in_=pt_ext[:])
            bt_sb = const.tile([ER, O], FP)
            nc.sync.dma_start(out=bt_sb[:], in_=bt_ext[:])
            sel_sb = const.tile([E, ER], FP)
            nc.sync.dma_start(out=sel_sb[:], in_=sel_ext[:])

            # ---- Phase A: load x tiles, transpose to xt (chunk-major) ----
            # xt col layout: c * T_SHARD + t   (chunk c, token t)
            xt = xt_pool.tile([P, N_CHUNKS * T_SHARD], FP)
            xt_v = xt.rearrange("p (c t) -> p c t", c=N_CHUNKS)
            for i in range(N_TILES):
                xin = xin_pool.tile([P, F], FP, tag="xin")
                nc.sync.dma_start(out=xin[:], in_=x_ext[i * P : (i + 1) * P, :])
                for q in range(N_CHUNKS // 4):
                    tp = tpd_psum.tile([P, 512], FP, tag="tp")
                    for cc in range(4):
                        c = q * 4 + cc
                        nc.tensor.transpose(
                            tp[:, cc * P : (cc + 1) * P],
                            xin[:, c * P : (c + 1) * P],
                            ident,
                        )
                    nc.vector.tensor_copy(
                        out=xt_v[:, q * 4 : (q + 1) * 4, i * P : (i + 1) * P],
                        in_=tp.rearrange("p (c t) -> p c t", c=4),
                    )

            # ---- Phase B: z (A-proj) + sim accumulation over K chunks ----
            z_ps = z_pool.tile([P, T_SHARD], FP)       # [er, t], 2 banks
            s_ps = s_pool.tile([E, T_SHARD], FP)       # [e, t], 2 banks
            for c in range(N_CHUNKS):
                for h in range(2):
                    rhs = xt[:, c * T_SHARD + h * 512 : c * T_SHARD + (h + 1) * 512]
                    nc.tensor.matmul(
                        z_ps[:, h * 512 : (h + 1) * 512],
                        lhsT=at_sb[:, c * P : (c + 1) * P],
                        rhs=rhs,
                        start=(c == 0),
                        stop=(c == N_CHUNKS - 1),
                    )
                    nc.tensor.matmul(
                        s_ps[:, h * 512 : (h + 1) * 512],
                        lhsT=pt_sb[:, c * E : (c + 1) * E],
                        rhs=rhs,
                        start=(c == 0),
                        stop=(c == N_CHUNKS - 1),
                    )

            # ---- Phase C: routing + weighting + B-matmul per token tile ----
            simabs = const.tile([E, T_SHARD], FP)
            nc.scalar.activation(simabs[:], s_ps[:], AF.Abs)

            for i in range(N_TILES):
                # sim tile -> [tok, E]
                sa_p = small_psum.tile([P, E], FP, tag="ps_small")
                nc.tensor.transpose(
                    sa_p[:], simabs[:, i * P : (i + 1) * P], ident[:E, :E]
                )
                sa = rt_pool.tile([P, E], FP, tag="sa")
                nc.vector.tensor_copy(sa[:], sa_p[:])

                # top-8 (sorted desc); m1 = col0, m2 = col1
                m8 = rt_pool.tile([P, 8], FP, tag="m8")
                nc.vector.max(out=m8[:], in_=sa[:])
                negm1 = rt_pool.tile([P, 1], FP, tag="negm1")
                nc.vector.tensor_scalar_mul(negm1[:], m8[:, 0:1], -1.0)
                exps = rt_pool.tile([P, E], FP, tag="exps")
                nc.scalar.activation(exps[:], sa[:], AF.Exp, bias=negm1[:], scale=1.0)
                mask = rt_pool.tile([P, E], FP, tag="mask")
                nc.vector.tensor_tensor(
                    mask[:], sa[:], m8[:, 1:2].to_broadcast([P, E]), op=ALU.is_ge
                )
                masked = rt_pool.tile([P, E], FP, tag="masked")
                nc.vector.tensor_tensor(masked[:], exps[:], mask[:], op=ALU.mult)
                denom = rt_pool.tile([P, 1], FP, tag="denom")
                nc.vector.reduce_sum(denom[:], masked[:], axis=AX.X)
                rec = rt_pool.tile([P, 1], FP, tag="rec")
                nc.vector.reciprocal(rec[:], denom[:])
                coeff = rt_pool.tile([P, E], FP, tag="coeff")
                nc.vector.tensor_tensor(
                    coeff[:], masked[:], rec.to_broadcast([P, E]), op=ALU.mult
                )

                # coeff [tok, E] -> ct [E, tok] -> broadcast to [er, tok]
                ct_p = small_psum.tile([E, P], FP, tag="ps_small")
                nc.tensor.transpose(ct_p[:], coeff[:], ident)
                ct = rt_pool.tile([E, P], FP, tag="ct")
                nc.vector.tensor_copy(ct[:], ct_p[:])
                cw_p = small_psum.tile([P, P], FP, tag="ps_small")
                nc.tensor.matmul(cw_p[:], lhsT=sel_sb[:], rhs=ct[:], start=True, stop=True)
                cwb = rt_pool.tile([P, P], FP, tag="cwb")
                nc.vector.tensor_copy(cwb[:], cw_p[:])

                # W[er, t] = z[er, t] * cwb[er, t]
                w_i = rt_pool.tile([P, P], FP, tag="w")
                nc.vector.tensor_tensor(
                    w_i[:], z_ps[:, i * P : (i + 1) * P], cwb[:], op=ALU.mult
                )

                # delta[t, :] = W.T @ BT
                osb = out_pool.tile([P, O], FP, tag="osb")
                for n in range(4):
                    dp = tpd_psum.tile([P, 512], FP, tag="tp")
                    nc.tensor.matmul(
                        dp[:],
                        lhsT=w_i[:],
                        rhs=bt_sb[:, n * 512 : (n + 1) * 512],
                        start=True,
                        stop=True,
                    )
                    nc.vector.tensor_copy(osb[:, n * 512 : (n + 1) * 512], dp[:])
                nc.sync.dma_start(out=out_ext[i * P : (i + 1) * P, :], in_=osb[:])

    _split_multi_waits(nc)
    return nc


def _prep_weights(prototypes, A_stack, B_stack, scales):
    # at: lhsT chunks for the A-projection. at[p, c*128+m] = A_all[m, c*128+p]
    A_all = A_stack.reshape(ER, F)                       # [128, 2048]
    at = np.ascontiguousarray(
        A_all.T.reshape(N_CHUNKS, P, P).transpose(1, 0, 2).reshape(P, N_CHUNKS * P)
    ).astype(np.float32)
    # pt: protos.T chunks. pt[p, c*8+e] = protos[e, c*128+p]
    pt = np.ascontiguousarray(
        prototypes.T.reshape(N_CHUNKS, P, E).transpose(1, 0, 2).reshape(P, N_CHUNKS * E)
    ).astype(np.float32)
    # bt: [er, O] with scales folded in
    bt = np.ascontiguousarray(
        (B_stack * scales[:, None, None]).transpose(0, 2, 1).reshape(ER, O)
    ).astype(np.float32)
    # sel: [E, ER] block-broadcast selector
    sel = np.zeros((E, ER), dtype=np.float32)
    for e in range(E):
        sel[e, e * R : (e + 1) * R] = 1.0
    return at, pt, bt, sel


_LAST_RESULT = {}


def kernel(x, prototypes, A_stack, B_stack, scales, top_k, _trace=False):
    assert int(top_k) == 2
    x = np.asarray(x, dtype=np.float32)
    B, S, _ = x.shape
    tok = x.reshape(-1, F)
    t_total = tok.shape[0]
    assert t_total == N_CORES * T_SHARD

    at, pt, bt, sel = _prep_weights(
        np.asarray(prototypes, np.float32),
        np.asarray(A_stack, np.float32),
        np.asarray(B_stack, np.float32),
        np.asarray(scales, np.float32),
    )

    nc = build_nc()

    in_maps = []
    for i in range(N_CORES):
        shard = np.ascontiguousarray(tok[i * T_SHARD : (i + 1) * T_SHARD])
        in_maps.append({"x": shard, "at": at, "pt": pt, "bt": bt, "sel": sel})

    res = run_bass_kernel_spmd(
        nc, in_maps, core_ids=list(range(N_CORES)), trace=_trace
    )
    _LAST_RESULT["exec_time_ns"] = res.exec_time_ns
    _LAST_RESULT["results"] = res

    out = np.concatenate([res.results[i]["out"] for i in range(N_CORES)], axis=0)
    return out.reshape(B, S, O)


if __name__ == "__main__":
    rng = np.random.default_rng(0)
    x = rng.standard_normal((4, 2048, 2048), dtype=np.float32)
    protos = rng.standard_normal((8, 2048)).astype(np.float32)
    protos /= np.linalg.norm(protos, axis=-1, keepdims=True) + 1e-8
    A = (rng.standard_normal((8, 16, 2048)) * 0.02).astype(np.float32)
    Bm = (rng.standard_normal((8, 2048, 16)) * 0.02).astype(np.float32)
    sc = rng.random(8).astype(np.float32)
    y = kernel(x, protos, A, Bm, sc, 2)
    print("out", y.shape, y.dtype, float(np.abs(y).mean()))


# revision 16
# speedup vs baseline: 1.5836x; 1.5836x over previous
"""Trainium2 Bass kernel for ArrowLoraLinearLayer (MoE top-2 LoRA routing).

Math (per token t):
  sim[t,e]  = |x[t,:] @ protos[e,:]|                      (E=8 experts)
  coeff     = softmax over top-2 of sim (others 0)
  z[t,:]    = x[t,:] @ A_all.T          A_all = [E*r, F]  (E*r = 128)
  W[er,t]   = coeff[t,e(er)] * z[t,er]
  out[t,:]  = W[:,t].T @ BT             BT[er,:] = scales[e] * B_stack[e,:,j].T

Sharding: data-parallel over tokens, 1024 tokens per core x 8 cores.
All weights replicated. No collectives.

Matmuls run in float32r (hw fp32 fast mode: bf16 hi/lo decomposition,
~16-bit mantissa, 4x the throughput of plain fp32 on the PE array).
"""

import sys
import types

sys.path.insert(0, "/opt/trn_rl_repo")

import numpy as np


def _install_ntff_hook_shim():
    """The agent image's antenv lacks axon_hooks; provide it so
    run_bass_kernel_spmd(trace=True) can profile via the axon .so."""
    if "antenv.axon_hooks" in sys.modules:
        return
    mod = types.ModuleType("antenv.axon_hooks")
    state = {"hook": None}

    def set_axon_ntff_profile_hook(h):
        state["hook"] = h

    def get_axon_ntff_profile_hook():
        if state["hook"] is None:
            try:
                from trn_agent_boot.trn_boot import _ntff_profile_via_ctypes

                state["hook"] = _ntff_profile_via_ctypes(
                    "/opt/axon/libaxon_pjrt.so"
                )
            except Exception:
                return None
        return state["hook"]

    mod.set_axon_ntff_profile_hook = set_axon_ntff_profile_hook
    mod.get_axon_ntff_profile_hook = get_axon_ntff_profile_hook
    sys.modules["antenv.axon_hooks"] = mod


_install_ntff_hook_shim()

import concourse.bass as bass
import concourse.mybir as mybir
from concourse.bass_utils import run_bass_kernel_spmd
from concourse.masks import make_identity
from concourse.tile import TileContext


def _split_multi_waits(nc, skip_opcodes=()):
    """Walrus allows only one sync-wait per engine instruction (e.g. the
    Matmult LDWEIGHTS slot, DMA_DIRECT2D). Move extra waits onto freshly
    inserted same-engine NoOps just before the instruction."""
    counter = 0
    for f in nc.m.functions:
        for b in f.blocks:
            il = b.instructions
            i = 0
            while i < len(il):
                inst = il[i]
                si = getattr(inst, "sync_info", None)
                if (
                    si is not None
                    and getattr(inst, "opcode", None) not in skip_opcodes
                    and len(si.on_wait) >= 2
                ):
                    waits = list(si.on_wait)
                    for w in waits:
                        nop = mybir.InstNoOp(name=f"I-waitsplit-{counter}")
                        counter += 1
                        nop.engine = inst.engine
                        nop.sync_info = mybir.SyncInfo(on_wait=[w], on_update=[])
                        il.insert(i, nop)
                        i += 1
                    inst.sync_info = mybir.SyncInfo(
                        on_wait=[], on_update=si.on_update
                    )
                i += 1


N_CORES = 8
P = 128            # partitions
F = 2048           # in features
O = 2048           # out features
E = 8              # experts
R = 16             # lora rank
ER = E * R         # 128
T_SHARD = 1024     # tokens per core
N_TILES = T_SHARD // P   # 8 token tiles per core
N_CHUNKS = F // P        # 16 K-chunks
FP = mybir.dt.float32
FPR = mybir.dt.float32r

AF = mybir.ActivationFunctionType
ALU = mybir.AluOpType
AX = mybir.AxisListType


def build_nc(sim_f32r=True):
    def _s(ap):
        # sim-path operand: fp32r (fast) or exact fp32 via bitcast
        return ap if sim_f32r else ap.bitcast(FP)

    nc = bass.Bass(target_bir_lowering=False)

    x_ext = nc.declare_dram_parameter("x", [T_SHARD, F], FPR, isOutput=False)
    at_ext = nc.declare_dram_parameter("at", [P, N_CHUNKS * P], FPR, isOutput=False)
    pt_ext = nc.declare_dram_parameter("pt", [P, N_CHUNKS * E], FPR, isOutput=False)
    bt_ext = nc.declare_dram_parameter("bt", [ER, O], FPR, isOutput=False)
    sel_ext = nc.declare_dram_parameter("sel", [E, ER], FPR, isOutput=False)
    out_ext = nc.declare_dram_parameter("out", [T_SHARD, O], FP, isOutput=True)

    with TileContext(nc) as tc:
        with (
            tc.tile_pool(name="const", bufs=1) as const,
            tc.tile_pool(name="xin", bufs=3) as xin_pool,
            tc.tile_pool(name="xt", bufs=1) as xt_pool,
            tc.tile_pool(name="rt", bufs=2) as rt_pool,
            tc.tile_pool(name="outp", bufs=2) as out_pool,
            tc.tile_pool(name="tpd", bufs=2, space="PSUM") as tpd_psum,
            tc.tile_pool(name="zp", bufs=1, space="PSUM") as z_pool,
            tc.tile_pool(name="sp", bufs=1, space="PSUM") as s_pool,
            tc.tile_pool(name="smallp", bufs=2, space="PSUM") as small_psum,
        ):
            ident32 = const.tile([P, P], FP)
            make_identity(nc, ident32)
            ident = const.tile([P, P], FPR)
            nc.vector.tensor_copy(ident[:], ident32[:])

            at_sb = const.tile([P, N_CHUNKS * P], FPR)
            nc.sync.dma_start(out=at_sb[:], in_=at_ext[:])
            pt_sb = const.tile([P, N_CHUNKS * E], FPR)
            nc.sync.dma_start(out=pt_sb[:], in_=pt_ext[:])
            bt_sb = const.tile([ER, O], FPR)
            nc.sync.dma_start(out=bt_sb[:], in_=bt_ext[:])
            sel_sb = const.tile([E, ER], FPR)
            nc.sync.dma_start(out=sel_sb[:], in_=sel_ext[:])

            # ---- Phase A: load x tiles, transpose to xt (chunk-major) ----
            # xt col layout: c * T_SHARD + t   (chunk c, token t)
            xt = xt_pool.tile([P, N_CHUNKS * T_SHARD], FPR)
            xt_v = xt.rearrange("p (c t) -> p c t", c=N_CHUNKS)
            for i in range(N_TILES):
                xin = xin_pool.tile([P, F], FPR, tag="xin")
                nc.sync.dma_start(out=xin[:], in_=x_ext[i * P : (i + 1) * P, :])
                for q in range(N_CHUNKS // 4):
                    tp = tpd_psum.tile([P, 512], FP, tag="tp")
                    for cc in range(4):
                        c = q * 4 + cc
                        nc.tensor.transpose(
                            tp[:, cc * P : (cc + 1) * P].bitcast(FPR),
                            xin[:, c * P : (c + 1) * P],
                            ident[:],
                        )
                    nc.vector.tensor_copy(
                        out=xt_v[:, q * 4 : (q + 1) * 4, i * P : (i + 1) * P],
                        in_=tp.rearrange("p (c t) -> p c t", c=4),
                    )

            # ---- Phase B: z (A-proj) + sim accumulation over K chunks ----
            z_ps = z_pool.tile([P, T_SHARD], FP)       # [er, t], 2 banks
            s_ps = s_pool.tile([E, T_SHARD], FP)       # [e, t], 2 banks
            for c in range(N_CHUNKS):
                for h in range(2):
                    rhs = xt[:, c * T_SHARD + h * 512 : c * T_SHARD + (h + 1) * 512]
                    nc.tensor.matmul(
                        z_ps[:, h * 512 : (h + 1) * 512],
                        lhsT=at_sb[:, c * P : (c + 1) * P],
                        rhs=rhs,
                        start=(c == 0),
                        stop=(c == N_CHUNKS - 1),
                    )
                    nc.tensor.matmul(
                        s_ps[:, h * 512 : (h + 1) * 512],
                        lhsT=_s(pt_sb[:, c * E : (c + 1) * E]),
                        rhs=_s(rhs),
                        start=(c == 0),
                        stop=(c == N_CHUNKS - 1),
                    )

            # ---- Phase C: routing + weighting + B-matmul per token tile ----
            simabs = const.tile([E, T_SHARD], FP)
            nc.scalar.activation(simabs[:], s_ps[:], AF.Abs)

            for i in range(N_TILES):
                # sim tile -> [tok, E]
                sa_p = small_psum.tile([P, E], FP, tag="ps_small")
                nc.tensor.transpose(
                    sa_p[:],
                    simabs[:, i * P : (i + 1) * P],
                    ident[:E, :E].bitcast(FP),
                )
                sa = rt_pool.tile([P, E], FP, tag="sa")
                nc.vector.tensor_copy(sa[:], sa_p[:])

                # top-8 (sorted desc); m1 = col0, m2 = col1
                m8 = rt_pool.tile([P, 8], FP, tag="m8")
                nc.vector.max(out=m8[:], in_=sa[:])
                negm1 = rt_pool.tile([P, 1], FP, tag="negm1")
                nc.vector.tensor_scalar_mul(negm1[:], m8[:, 0:1], -1.0)
                exps = rt_pool.tile([P, E], FP, tag="exps")
                nc.scalar.activation(exps[:], sa[:], AF.Exp, bias=negm1[:], scale=1.0)
                mask = rt_pool.tile([P, E], FP, tag="mask")
                nc.vector.tensor_tensor(
                    mask[:], sa[:], m8[:, 1:2].to_broadcast([P, E]), op=ALU.is_ge
                )
                masked = rt_pool.tile([P, E], FP, tag="masked")
                nc.vector.tensor_tensor(masked[:], exps[:], mask[:], op=ALU.mult)
                denom = rt_pool.tile([P, 1], FP, tag="denom")
                nc.vector.reduce_sum(denom[:], masked[:], axis=AX.X)
                rec = rt_pool.tile([P, 1], FP, tag="rec")
                nc.vector.reciprocal(rec[:], denom[:])
                coeff = rt_pool.tile([P, E], FP, tag="coeff")
                nc.vector.tensor_tensor(
                    coeff[:], masked[:], rec.to_broadcast([P, E]), op=ALU.mult
                )

                # coeff [tok, E] -> ct [E, tok] -> broadcast to [er, tok]
                ct_p = small_psum.tile([E, P], FP, tag="ps_small")
                nc.tensor.transpose(ct_p[:], coeff[:], ident.bitcast(FP))
                ct = rt_pool.tile([E, P], FPR, tag="ct")
                nc.vector.tensor_copy(ct[:], ct_p[:])
                cw_p = small_psum.tile([P, P], FP, tag="ps_small")
                nc.tensor.matmul(cw_p[:], lhsT=sel_sb[:], rhs=ct[:], start=True, stop=True)
                cwb = rt_pool.tile([P, P], FP, tag="cwb")
                nc.vector.tensor_copy(cwb[:], cw_p[:])

                # W[er, t] = z[er, t] * cwb[er, t]  (rounded to f32r on write)
                w_i = rt_pool.tile([P, P], FPR, tag="w")
                nc.vector.tensor_tensor(
                    w_i[:], z_ps[:, i * P : (i + 1) * P], cwb[:], op=ALU.mult
                )

                # delta[t, :] = W.T @ BT
                osb = out_pool.tile([P, O], FP, tag="osb")
                for n in range(4):
                    dp = tpd_psum.tile([P, 512], FP, tag="tp")
                    nc.tensor.matmul(
                        dp[:],
                        lhsT=w_i[:],
                        rhs=bt_sb[:, n * 512 : (n + 1) * 512],
                        start=True,
                        stop=True,
                    )
                    # delta evictions on the (otherwise idle) scalar engine
                    nc.scalar.activation(
                        osb[:, n * 512 : (n + 1) * 512], dp[:], AF.Copy
                    )
                nc.sync.dma_start(out=out_ext[i * P : (i + 1) * P, :], in_=osb[:])

    _split_multi_waits(nc)
    return nc


def _prep_weights(prototypes, A_stack, B_stack, scales):
    # at: lhsT chunks for the A-projection. at[p, c*128+m] = A_all[m, c*128+p]
    A_all = A_stack.reshape(ER, F)                       # [128, 2048]
    at = np.ascontiguousarray(
        A_all.T.reshape(N_CHUNKS, P, P).transpose(1, 0, 2).reshape(P, N_CHUNKS * P)
    ).astype(np.float32)
    # pt: protos.T chunks. pt[p, c*8+e] = protos[e, c*128+p]
    pt = np.ascontiguousarray(
        prototypes.T.reshape(N_CHUNKS, P, E).transpose(1, 0, 2).reshape(P, N_CHUNKS * E)
    ).astype(np.float32)
    # bt: [er, O] with scales folded in
    bt = np.ascontiguousarray(
        (B_stack * scales[:, None, None]).transpose(0, 2, 1).reshape(ER, O)
    ).astype(np.float32)
    # sel: [E, ER] block-broadcast selector
    sel = np.zeros((E, ER), dtype=np.float32)
    for e in range(E):
        sel[e, e * R : (e + 1) * R] = 1.0
    return at, pt, bt, sel


_LAST_RESULT = {}


def kernel(x, prototypes, A_stack, B_stack, scales, top_k, _trace=False, **_modes):
    assert int(top_k) == 2
    x = np.asarray(x, dtype=np.float32)
    B, S, _ = x.shape
    tok = x.reshape(-1, F)
    t_total = tok.shape[0]
    assert t_total == N_CORES * T_SHARD

    at, pt, bt, sel = _prep_weights(
        np.asarray(prototypes, np.float32),
        np.asarray(A_stack, np.float32),
        np.asarray(B_stack, np.float32),
        np.asarray(scales, np.float32),
    )

    nc = build_nc(**_modes)

    in_maps = []
    for i in range(N_CORES):
        shard = np.ascontiguousarray(tok[i * T_SHARD : (i + 1) * T_SHARD])
        in_maps.append({"x": shard, "at": at, "pt": pt, "bt": bt, "sel": sel})

    res = run_bass_kernel_spmd(
        nc, in_maps, core_ids=list(range(N_CORES)), trace=_trace
    )
    _LAST_RESULT["exec_time_ns"] = res.exec_time_ns
    _LAST_RESULT["results"] = res

    out = np.concatenate([res.results[i]["out"] for i in range(N_CORES)], axis=0)
    return out.reshape(B, S, O)


if __name__ == "__main__":
    rng = np.random.default_rng(0)
    x = rng.standard_normal((4, 2048, 2048), dtype=np.float32)
    protos = rng.standard_normal((8, 2048)).astype(np.float32)
    protos /= np.linalg.norm(protos, axis=-1, keepdims=True) + 1e-8
    A = (rng.standard_normal((8, 16, 2048)) * 0.02).astype(np.float32)
    Bm = (rng.standard_normal((8, 2048, 16)) * 0.02).astype(np.float32)
    sc = rng.random(8).astype(np.float32)
    y = kernel(x, protos, A, Bm, sc, 2)
    print("out", y.shape, y.dtype, float(np.abs(y).mean()))


# revision 17
# speedup vs baseline: 1.8747x; 1.1838x over previous
"""Trainium2 Bass kernel for ArrowLoraLinearLayer (MoE top-2 LoRA routing).

Math (per token t):
  sim[t,e]  = |x[t,:] @ protos[e,:]|                      (E=8 experts)
  coeff     = softmax over top-2 of sim (others 0)
  z[t,:]    = x[t,:] @ A_all.T          A_all = [E*r, F]  (E*r = 128)
  W[er,t]   = coeff[t,e(er)] * z[t,er]
  out[t,:]  = W[:,t].T @ BT             BT[er,:] = scales[e] * B_stack[e,:,j].T

Sharding: data-parallel over tokens, 1024 tokens per core x 8 cores.
All weights replicated. No collectives.

Matmuls run in float32r (hw fp32 fast mode: bf16 hi/lo decomposition,
~16-bit mantissa, 4x the throughput of plain fp32 on the PE array).
"""

import sys
import types

sys.path.insert(0, "/opt/trn_rl_repo")

import numpy as np


def _install_ntff_hook_shim():
    """The agent image's antenv lacks axon_hooks; provide it so
    run_bass_kernel_spmd(trace=True) can profile via the axon .so."""
    if "antenv.axon_hooks" in sys.modules:
        return
    mod = types.ModuleType("antenv.axon_hooks")
    state = {"hook": None}

    def set_axon_ntff_profile_hook(h):
        state["hook"] = h

    def get_axon_ntff_profile_hook():
        if state["hook"] is None:
            try:
                from trn_agent_boot.trn_boot import _ntff_profile_via_ctypes

                state["hook"] = _ntff_profile_via_ctypes(
                    "/opt/axon/libaxon_pjrt.so"
                )
            except Exception:
                return None
        return state["hook"]

    mod.set_axon_ntff_profile_hook = set_axon_ntff_profile_hook
    mod.get_axon_ntff_profile_hook = get_axon_ntff_profile_hook
    sys.modules["antenv.axon_hooks"] = mod


_install_ntff_hook_shim()

import concourse.bass as bass
import concourse.mybir as mybir
from concourse.bass_utils import run_bass_kernel_spmd
from concourse.masks import make_identity
from concourse.tile import TileContext


def _split_multi_waits(nc, skip_opcodes=()):
    """Walrus allows only one sync-wait per engine instruction (e.g. the
    Matmult LDWEIGHTS slot, DMA_DIRECT2D). Move extra waits onto freshly
    inserted same-engine NoOps just before the instruction."""
    counter = 0
    for f in nc.m.functions:
        for b in f.blocks:
            il = b.instructions
            i = 0
            while i < len(il):
                inst = il[i]
                si = getattr(inst, "sync_info", None)
                if (
                    si is not None
                    and getattr(inst, "opcode", None) not in skip_opcodes
                    and len(si.on_wait) >= 2
                ):
                    waits = list(si.on_wait)
                    for w in waits:
                        nop = mybir.InstNoOp(name=f"I-waitsplit-{counter}")
                        counter += 1
                        nop.engine = inst.engine
                        nop.sync_info = mybir.SyncInfo(on_wait=[w], on_update=[])
                        il.insert(i, nop)
                        i += 1
                    inst.sync_info = mybir.SyncInfo(
                        on_wait=[], on_update=si.on_update
                    )
                i += 1


N_CORES = 8
P = 128            # partitions
F = 2048           # in features
O = 2048           # out features
E = 8              # experts
R = 16             # lora rank
ER = E * R         # 128
T_SHARD = 1024     # tokens per core
N_TILES = T_SHARD // P   # 8 token tiles per core
N_CHUNKS = F // P        # 16 K-chunks
FP = mybir.dt.float32
FPR = mybir.dt.float32r
BF = mybir.dt.bfloat16

AF = mybir.ActivationFunctionType
ALU = mybir.AluOpType
AX = mybir.AxisListType


def build_nc(sim_f32r=True):
    def _s(ap):
        # sim-path operand: fp32r (fast) or exact fp32 via bitcast
        return ap if sim_f32r else ap.bitcast(FP)

    nc = bass.Bass(target_bir_lowering=False)

    x_ext = nc.declare_dram_parameter("x", [T_SHARD, F], FPR, isOutput=False)
    at_ext = nc.declare_dram_parameter("at", [P, N_CHUNKS * P], FPR, isOutput=False)
    pt_ext = nc.declare_dram_parameter("pt", [P, N_CHUNKS * E], FPR, isOutput=False)
    bt_ext = nc.declare_dram_parameter("bt", [ER, O], FPR, isOutput=False)
    sel_ext = nc.declare_dram_parameter("sel", [E, ER], FPR, isOutput=False)
    btb_ext = nc.declare_dram_parameter("btb", [ER, O], BF, isOutput=False)
    out_ext = nc.declare_dram_parameter("out", [T_SHARD, O], FP, isOutput=True)

    with TileContext(nc) as tc:
        with (
            tc.tile_pool(name="const", bufs=1) as const,
            tc.tile_pool(name="xin", bufs=3) as xin_pool,
            tc.tile_pool(name="xt", bufs=1) as xt_pool,
            tc.tile_pool(name="rt", bufs=2) as rt_pool,
            tc.tile_pool(name="outp", bufs=2) as out_pool,
            tc.tile_pool(name="tpd", bufs=2, space="PSUM") as tpd_psum,
            tc.tile_pool(name="zp", bufs=1, space="PSUM") as z_pool,
            tc.tile_pool(name="sp", bufs=1, space="PSUM") as s_pool,
            tc.tile_pool(name="smallp", bufs=2, space="PSUM") as small_psum,
        ):
            ident32 = const.tile([P, P], FP)
            make_identity(nc, ident32)
            ident = const.tile([P, P], FPR)
            nc.vector.tensor_copy(ident[:], ident32[:])

            at_sb = const.tile([P, N_CHUNKS * P], FPR)
            nc.sync.dma_start(out=at_sb[:], in_=at_ext[:])
            pt_sb = const.tile([P, N_CHUNKS * E], FPR)
            nc.sync.dma_start(out=pt_sb[:], in_=pt_ext[:])
            bt_sb = const.tile([ER, O], FPR)
            nc.sync.dma_start(out=bt_sb[:], in_=bt_ext[:])
            btb_sb = const.tile([ER, O], BF)
            nc.sync.dma_start(out=btb_sb[:], in_=btb_ext[:])
            sel_sb = const.tile([E, ER], FPR)
            nc.sync.dma_start(out=sel_sb[:], in_=sel_ext[:])

            # ---- Phase A: load x tiles, transpose to xt (chunk-major) ----
            # xt col layout: c * T_SHARD + t   (chunk c, token t)
            xt = xt_pool.tile([P, N_CHUNKS * T_SHARD], FPR)
            xt_v = xt.rearrange("p (c t) -> p c t", c=N_CHUNKS)
            for i in range(N_TILES):
                xin = xin_pool.tile([P, F], FPR, tag="xin")
                nc.sync.dma_start(out=xin[:], in_=x_ext[i * P : (i + 1) * P, :])
                for q in range(N_CHUNKS // 4):
                    tp = tpd_psum.tile([P, 512], FP, tag="tp")
                    for cc in range(4):
                        c = q * 4 + cc
                        nc.tensor.transpose(
                            tp[:, cc * P : (cc + 1) * P].bitcast(FPR),
                            xin[:, c * P : (c + 1) * P],
                            ident[:],
                        )
                    nc.vector.tensor_copy(
                        out=xt_v[:, q * 4 : (q + 1) * 4, i * P : (i + 1) * P],
                        in_=tp.rearrange("p (c t) -> p c t", c=4),
                    )

            # ---- Phase B: z (A-proj) + sim accumulation over K chunks ----
            z_ps = z_pool.tile([P, T_SHARD], FP)       # [er, t], 2 banks
            s_ps = s_pool.tile([E, T_SHARD], FP)       # [e, t], 2 banks
            for c in range(N_CHUNKS):
                for h in range(2):
                    rhs = xt[:, c * T_SHARD + h * 512 : c * T_SHARD + (h + 1) * 512]
                    nc.tensor.matmul(
                        z_ps[:, h * 512 : (h + 1) * 512],
                        lhsT=at_sb[:, c * P : (c + 1) * P],
                        rhs=rhs,
                        start=(c == 0),
                        stop=(c == N_CHUNKS - 1),
                    )
                    nc.tensor.matmul(
                        s_ps[:, h * 512 : (h + 1) * 512],
                        lhsT=_s(pt_sb[:, c * E : (c + 1) * E]),
                        rhs=_s(rhs),
                        start=(c == 0),
                        stop=(c == N_CHUNKS - 1),
                    )

            # ---- Phase C: routing + weighting + B-matmul per token tile ----
            simabs = const.tile([E, T_SHARD], FP)
            nc.scalar.activation(simabs[:], s_ps[:], AF.Abs)

            for i in range(N_TILES):
                # sim tile -> [tok, E]
                sa_p = small_psum.tile([P, E], FP, tag="ps_small")
                nc.tensor.transpose(
                    sa_p[:],
                    simabs[:, i * P : (i + 1) * P],
                    ident[:E, :E].bitcast(FP),
                )
                sa = rt_pool.tile([P, E], FP, tag="sa")
                nc.vector.tensor_copy(sa[:], sa_p[:])

                # top-8 (sorted desc); m1 = col0, m2 = col1
                m8 = rt_pool.tile([P, 8], FP, tag="m8")
                nc.vector.max(out=m8[:], in_=sa[:])
                negm1 = rt_pool.tile([P, 1], FP, tag="negm1")
                nc.vector.tensor_scalar_mul(negm1[:], m8[:, 0:1], -1.0)
                exps = rt_pool.tile([P, E], FP, tag="exps")
                nc.scalar.activation(exps[:], sa[:], AF.Exp, bias=negm1[:], scale=1.0)
                mask = rt_pool.tile([P, E], FP, tag="mask")
                nc.vector.tensor_tensor(
                    mask[:], sa[:], m8[:, 1:2].to_broadcast([P, E]), op=ALU.is_ge
                )
                masked = rt_pool.tile([P, E], FP, tag="masked")
                nc.vector.tensor_tensor(masked[:], exps[:], mask[:], op=ALU.mult)
                denom = rt_pool.tile([P, 1], FP, tag="denom")
                nc.vector.reduce_sum(denom[:], masked[:], axis=AX.X)
                rec = rt_pool.tile([P, 1], FP, tag="rec")
                nc.vector.reciprocal(rec[:], denom[:])
                coeff = rt_pool.tile([P, E], FP, tag="coeff")
                nc.vector.tensor_tensor(
                    coeff[:], masked[:], rec.to_broadcast([P, E]), op=ALU.mult
                )

                # coeff [tok, E] -> ct [E, tok] -> broadcast to [er, tok]
                ct_p = small_psum.tile([E, P], FP, tag="ps_small")
                nc.tensor.transpose(ct_p[:], coeff[:], ident.bitcast(FP))
                ct = rt_pool.tile([E, P], FPR, tag="ct")
                nc.vector.tensor_copy(ct[:], ct_p[:])
                cw_p = small_psum.tile([P, P], FP, tag="ps_small")
                nc.tensor.matmul(cw_p[:], lhsT=sel_sb[:], rhs=ct[:], start=True, stop=True)
                cwb = rt_pool.tile([P, P], FP, tag="cwb")
                nc.vector.tensor_copy(cwb[:], cw_p[:])

                # W[er, t] = z[er, t] * cwb[er, t]  (rounded to f32r on write)
                w_i = rt_pool.tile([P, P], BF, tag="w")
                nc.vector.tensor_tensor(
                    w_i[:], z_ps[:, i * P : (i + 1) * P], cwb[:], op=ALU.mult
                )

                # delta[t, :] = W.T @ BT
                osb = out_pool.tile([P, O], FP, tag="osb")
                for n in range(4):
                    dp = tpd_psum.tile([P, 512], FP, tag="tp")
                    nc.tensor.matmul(
                        dp[:],
                        lhsT=w_i[:],
                        rhs=btb_sb[:, n * 512 : (n + 1) * 512],
                        start=True,
                        stop=True,
                    )
                    # delta evictions on the (otherwise idle) scalar engine
                    nc.scalar.activation(
                        osb[:, n * 512 : (n + 1) * 512], dp[:], AF.Copy
                    )
                nc.sync.dma_start(out=out_ext[i * P : (i + 1) * P, :], in_=osb[:])

    _split_multi_waits(nc)
    return nc


def _prep_weights(prototypes, A_stack, B_stack, scales):
    # at: lhsT chunks for the A-projection. at[p, c*128+m] = A_all[m, c*128+p]
    A_all = A_stack.reshape(ER, F)                       # [128, 2048]
    at = np.ascontiguousarray(
        A_all.T.reshape(N_CHUNKS, P, P).transpose(1, 0, 2).reshape(P, N_CHUNKS * P)
    ).astype(np.float32)
    # pt: protos.T chunks. pt[p, c*8+e] = protos[e, c*128+p]
    pt = np.ascontiguousarray(
        prototypes.T.reshape(N_CHUNKS, P, E).transpose(1, 0, 2).reshape(P, N_CHUNKS * E)
    ).astype(np.float32)
    # bt: [er, O] with scales folded in
    bt = np.ascontiguousarray(
        (B_stack * scales[:, None, None]).transpose(0, 2, 1).reshape(ER, O)
    ).astype(np.float32)
    # sel: [E, ER] block-broadcast selector
    sel = np.zeros((E, ER), dtype=np.float32)
    for e in range(E):
        sel[e, e * R : (e + 1) * R] = 1.0
    import ml_dtypes
    btb = bt.astype(ml_dtypes.bfloat16)
    return at, pt, bt, sel, btb


_LAST_RESULT = {}


def kernel(x, prototypes, A_stack, B_stack, scales, top_k, _trace=False, **_modes):
    assert int(top_k) == 2
    x = np.asarray(x, dtype=np.float32)
    B, S, _ = x.shape
    tok = x.reshape(-1, F)
    t_total = tok.shape[0]
    assert t_total == N_CORES * T_SHARD

    at, pt, bt, sel, btb = _prep_weights(
        np.asarray(prototypes, np.float32),
        np.asarray(A_stack, np.float32),
        np.asarray(B_stack, np.float32),
        np.asarray(scales, np.float32),
    )

    nc = build_nc(**_modes)

    in_maps = []
    for i in range(N_CORES):
        shard = np.ascontiguousarray(tok[i * T_SHARD : (i + 1) * T_SHARD])
        in_maps.append({"x": shard, "at": at, "pt": pt, "bt": bt, "sel": sel, "btb": btb})

    res = run_bass_kernel_spmd(
        nc, in_maps, core_ids=list(range(N_CORES)), trace=_trace
    )
    _LAST_RESULT["exec_time_ns"] = res.exec_time_ns
    _LAST_RESULT["results"] = res

    out = np.concatenate([res.results[i]["out"] for i in range(N_CORES)], axis=0)
    return out.reshape(B, S, O)


if __name__ == "__main__":
    rng = np.random.default_rng(0)
    x = rng.standard_normal((4, 2048, 2048), dtype=np.float32)
    protos = rng.standard_normal((8, 2048)).astype(np.float32)
    protos /= np.linalg.norm(protos, axis=-1, keepdims=True) + 1e-8
    A = (rng.standard_normal((8, 16, 2048)) * 0.02).astype(np.float32)
    Bm = (rng.standard_normal((8, 2048, 16)) * 0.02).astype(np.float32)
    sc = rng.random(8).astype(np.float32)
    y = kernel(x, protos, A, Bm, sc, 2)
    print("out", y.shape, y.dtype, float(np.abs(y).mean()))
